# revision 4
# baseline (speedup 1.0000x reference)
"""Causal multi-head attention on 8 trn2 NeuronCores (Megatron-style head parallelism).

Problem: B=2, L=2048, D=1024, H=16 heads (HD=64), fp32 in/out.

Sharding: each of the 8 cores owns 2 heads (a 128-wide slice of the QKV
projection output / Wo rows). Every core reads the full x; QKV projections are
column-sharded, attention runs per-head, the output projection is row-sharded
producing a partial sum per core which the host reduces (+ bo).

On-chip layout: activations are feature-major: x^T [D, B*L] (host
pre-transposes), Q^T/K^T/V^T [128(d), L] per batch. Scores are computed
transposed: S^T[k, q] = K_blk^T.T @ Q^T (contraction over head dim), exp on
the scalar engine, ctx^T[d, q] accumulates over key blocks with V-natural
(built via DMA-XBAR transpose into contiguous tiles) as the stationary
operand.

Perf structure (v2 — single interleaved PE stream):
  - everything is emitted as one continuous PE instruction stream per batch:
    attention steps carry "filler" work units (QKV projection / output
    projection matmul groups) injected between the score matmul of step i+1
    and the ctx matmul of step i.  The PE never waits on the scalar engine's
    exp (which covers its ~1us latency under the filler), stays p-state
    ramped, and the projection phases cost no extra wall-clock.
  - all PSUM work units allocate from the same 2-buf "sc" pool ring
    ([128,1024]f32 = 2 banks each; sc 4 banks + ctx 4 banks = 8 banks).
  - causal work trimmed at 128-col granularity on diagonal blocks; causal
    mask applied additively in PSUM via an identity-stationary matmul
    (value -1000 before the 1/8 softmax scale -> exp underflows to exact 0).
  - ctx packed [128d, L]: h0 ctx rows 0-64 of psum bank A (inline ones column
    gives the h0 softmax denominator in row 64), h1 ctx rows 64-127 of bank B
    with its denominator from a 1-col side-matmul into bank B row 32.
    Reciprocals via the fast DVE approx; the per-column broadcast is a rank-1
    f32r matmul into the ctx tile's free psum regions, emitted two score
    tiles into the next query tile so the PE never waits on the reciprocal.
  - output projection contracts all 128 dims in one matmul per (128-token
    block, half); interleaved into the SAME batch's attention one query tile
    behind the norm, so the tail is only the last query tile's outproj.
  - first x strip DMA'd in [64, 512] chunks ordered (half, ec, row) so the
    first projection matmul starts as soon as ~64KB has landed; weight DMAs
    split per 2 ec chunks.
"""

import numpy as np

_B, _L, _D, _H, _HD = 2, 2048, 1024, 16, 64
_NC = 8
_DC = _D // _NC          # 128 feature dims (2 heads) per core
_T = _B * _L             # 4096 tokens
_NKB = _L // 128         # 16 key blocks per batch
_NQT = _L // 512         # 4 query tiles per batch

_cache = {}


def _build_bass():
    from concourse import bacc
    import concourse.mybir as mybir
    import concourse.tile as tile

    f32 = mybir.dt.float32
    f16 = mybir.dt.float16
    bf16 = mybir.dt.bfloat16
    AFT = mybir.ActivationFunctionType

    nc = bacc.Bacc("TRN2", target_bir_lowering=False, debug=False, num_devices=_NC)

    xT = nc.dram_tensor("xT", [_D, _T], f16, kind="ExternalInput")
    wq = nc.dram_tensor("wq", [_D, _DC], f16, kind="ExternalInput")
    wk = nc.dram_tensor("wk", [_D, _DC], f16, kind="ExternalInput")
    wv = nc.dram_tensor("wv", [_D, _DC], f16, kind="ExternalInput")
    wo = nc.dram_tensor("wo", [_DC, _D], f16, kind="ExternalInput")
    bqd = nc.dram_tensor("bq", [_DC, 1], f32, kind="ExternalInput")
    bkd = nc.dram_tensor("bk", [_DC, 1], f32, kind="ExternalInput")
    bvd = nc.dram_tensor("bv", [_DC, 1], f32, kind="ExternalInput")
    idnd = nc.dram_tensor("idn", [128, 128], f16, kind="ExternalInput")
    mskd = nc.dram_tensor("msk", [128, 128], f16, kind="ExternalInput")
    onsd = nc.dram_tensor("ons", [128, _NKB], f16, kind="ExternalInput")
    zond = nc.dram_tensor("zon", [128, _NKB, 64], f16, kind="ExternalInput")
    onrd = nc.dram_tensor("onr", [128, 64], bf16, kind="ExternalInput")
    out = nc.dram_tensor("out", [_T, _D], f16, kind="ExternalOutput")

    with tile.TileContext(nc) as tc:
        with (
            tc.tile_pool(name="const", bufs=1) as constp,
            tc.tile_pool(name="xs", bufs=2) as xsp,
            tc.tile_pool(name="qkv", bufs=2) as qkvp,
            tc.tile_pool(name="pr", bufs=3) as prp,
            tc.tile_pool(name="nrm", bufs=2) as nrmp,
            tc.tile_pool(name="og", bufs=4) as ogp,
            tc.tile_pool(name="sc", bufs=2, space="PSUM") as scp,  # 2x[128,1024]f32 = 4 banks
            tc.tile_pool(name="cx", bufs=2, space="PSUM") as cxp,  # 2x[128,1024]f32 = 4 banks
        ):
            # ---- persistent constants ----
            # ordering matters: the first projection chain needs only wv, bv
            # and the first chunks of x, so those DMAs go first
            wv_sb = constp.tile([128, 8, 128], f16, tag="wv")
            for i in range(4):
                nc.sync.dma_start(
                    wv_sb[:, 2 * i:2 * i + 2, :],
                    wv[2 * i * 128:(2 * i + 2) * 128, :].rearrange(
                        "(c p) d -> p c d", p=128
                    ),
                )
            bv_sb = constp.tile([128, 1], f32, tag="bv")
            nc.sync.dma_start(bv_sb[:], bvd[:])

            def prefetch_x(b):
                t0 = b * _L
                xss = []
                for tb2 in range(_L // 1024):
                    xs = xsp.tile([128, 8, 1024], f16, tag="xs", name="xs")
                    c0 = t0 + tb2 * 1024
                    if b == 0 and tb2 == 0:
                        # fine chunks ordered (half, ec, row-half) so the
                        # first V-proj matmul starts after ~64KB lands
                        for half in range(2):
                            for ec in range(8):
                                for rh in range(2):
                                    nc.sync.dma_start(
                                        xs[64 * rh:64 * rh + 64, ec,
                                           half * 512:(half + 1) * 512],
                                        xT[ec * 128 + 64 * rh:
                                           ec * 128 + 64 * rh + 64,
                                           c0 + half * 512:c0 + (half + 1) * 512],
                                    )
                    else:
                        for i in range(2):
                            ec0 = i * 4
                            nc.sync.dma_start(
                                xs[:, ec0:ec0 + 4, :],
                                xT[ec0 * 128:(ec0 + 4) * 128,
                                   c0:c0 + 1024].rearrange(
                                    "(c p) t -> p c t", p=128
                                ),
                            )
                    xss.append(xs)
                return xss

            xss_b0 = prefetch_x(0)

            wq_sb = constp.tile([128, 8, 128], f16, tag="wq")
            wk_sb = constp.tile([128, 8, 128], f16, tag="wk")
            for w_sb, wd in ((wk_sb, wk), (wq_sb, wq)):
                for i in range(4):
                    nc.sync.dma_start(
                        w_sb[:, 2 * i:2 * i + 2, :],
                        wd[2 * i * 128:(2 * i + 2) * 128, :].rearrange(
                            "(c p) d -> p c d", p=128
                        ),
                    )
            bq_sb = constp.tile([128, 1], f32, tag="bq")
            bk_sb = constp.tile([128, 1], f32, tag="bk")
            nc.sync.dma_start(bk_sb[:], bkd[:])
            nc.sync.dma_start(bq_sb[:], bqd[:])
            wo_sb = constp.tile([128, 1024], f16, tag="wo")
            nc.sync.dma_start(wo_sb[:], wo[:])
            idn_sb = constp.tile([128, 128], f16, tag="idn")
            nc.sync.dma_start(idn_sb[:], idnd[:])
            msk_sb = constp.tile([128, 128], f16, tag="msk")
            nc.sync.dma_start(msk_sb[:], mskd[:])
            ons_sb = constp.tile([128, _NKB], f16, tag="ons")
            nc.sync.dma_start(ons_sb[:], onsd[:])
            onr_sb = constp.tile([128, 64], bf16, tag="onr")
            nc.sync.dma_start(onr_sb[:], onrd[:])

            # persistent V stationaries. v0 = [V0 | ones]: ctx rows 0-63 +
            # h0 denom row 64. v1e = [0..0 | ones@32 | 0..0 | V1]: one fused
            # matmul yields h1 denom at row 32 and ctx at rows 64-127.
            # Constant columns are written once; V parts repacked per batch.
            v0 = qkvp.tile([128, _NKB, 65], f16, tag="v0", name="v0", bufs=1)
            v1e = qkvp.tile([128, _NKB, 128], f16, tag="v1e", name="v1e", bufs=1)
            nc.vector.tensor_copy(v0[:, :, 64], ons_sb[:])
            nc.sync.dma_start(v1e[:, :, 0:64], zond[:])

            tls = {}

            def alloc_batch(b):
                qT = qkvp.tile([128, _L], f16, tag="qT", name="qT")
                kT = qkvp.tile([128, _L], f16, tag="kT", name="kT")
                vT = qkvp.tile([128, _L], f16, tag="vT", name="vT", bufs=1)
                ctx = qkvp.tile([128, _L], f16, tag="ctx", name="ctx")
                # V natural via DMA XBAR transpose (contiguous dests only),
                # then DVE re-pack into the strided stationary tiles.
                v0t = qkvp.tile([128, _NKB, 64], f16, tag="v0t", name="v0t", bufs=1)
                v1t = qkvp.tile([128, _NKB, 64], f16, tag="v1t", name="v1t", bufs=1)
                xss = xss_b0 if b == 0 else prefetch_x(b)
                tls[b] = dict(qT=qT, kT=kT, vT=vT, ctx=ctx, v0t=v0t, v1t=v1t,
                              xss=xss)
                return tls[b]

            def emit_proj_unit(b, tb2, w_sb, b_sb, dkey, is_v=False, is_q=False):
                """One strip x one projection: 16 matmuls + bias drain
                (+ V transpose/repack chain). ~3.4us of PE work."""
                tl = tls[b]
                xs = tl["xss"][tb2]
                dst = tl[dkey]
                ps = scp.tile([128, 1024], f32, tag="sc", name="pps")
                for half in range(2):
                    col = half * 512
                    for ec in range(8):
                        nc.tensor.matmul(
                            ps[:, col:col + 512],
                            w_sb[:, ec, :],
                            xs[:, ec, col:col + 512],
                            start=(ec == 0),
                            stop=(ec == 7),
                        )
                if is_q:
                    for bh in range(2):
                        bcol = tb2 * 1024 + bh * 512
                        nc.vector.tensor_scalar_add(
                            dst[:, bcol:bcol + 512],
                            ps[:, bh * 512:(bh + 1) * 512], b_sb[:]
                        )
                else:
                    nc.vector.tensor_scalar_add(
                        dst[:, tb2 * 1024:(tb2 + 1) * 1024], ps[:], b_sb[:]
                    )
                if is_v:
                    v0t, v1t = tl["v0t"], tl["v1t"]
                    for hh in range(2):
                        cs = slice(tb2 * 1024 + hh * 512,
                                   tb2 * 1024 + (hh + 1) * 512)
                        kbs = slice(tb2 * 8 + hh * 4, tb2 * 8 + hh * 4 + 4)
                        nc.sync.dma_start(
                            v0t[:, kbs, :], dst[0:64, cs], transpose=True
                        )
                        nc.sync.dma_start(
                            v1t[:, kbs, :], dst[64:128, cs], transpose=True
                        )
                        nc.vector.tensor_copy(v0[:, kbs, 0:64], v0t[:, kbs, :])
                        nc.vector.tensor_copy(v1e[:, kbs, 64:128], v1t[:, kbs, :])

            def emit_outproj_unit(b, tkb, eng):
                """Output projection for one 128-token block: 2 matmuls +
                staging copy + DMA. ~0.45us of PE work."""
                ctx = tls[b]["ctx"]
                op = scp.tile([128, 1024], f32, tag="sc", name="op")
                for half in range(2):
                    col = half * 512
                    nc.tensor.matmul(
                        op[:, col:col + 512],
                        ctx[:, tkb * 128:(tkb + 1) * 128],
                        wo_sb[:, col:col + 512],
                        start=True, stop=True,
                    )
                stg = ogp.tile([128, 1024], f16, tag="og", name="stg")
                if eng == "scalar":
                    nc.scalar.copy(stg[:], op[:])
                else:
                    nc.vector.tensor_copy(stg[:], op[:])
                r0 = b * _L + tkb * 128
                nc.sync.dma_start(out[r0:r0 + 128, :], stg[:])

            # ---- attention machinery ----
            def emit_sc(tl, qt, kb, nk, q0, cold):
                qT, kT = tl["qT"], tl["kT"]
                j = kb - (nk - 4)
                qlo = 128 * j if j > 0 else 0
                sc = scp.tile([128, 2, 512], f32, tag="sc", name="sct")
                for h in range(2):
                    hp = h * 64
                    nc.tensor.matmul(
                        sc[:, h, qlo:512],
                        kT[hp:hp + 64, kb * 128:(kb + 1) * 128],
                        qT[hp:hp + 64, q0 + qlo:q0 + 512],
                        start=True, stop=(j < 0),
                    )
                if j >= 0:
                    for h in range(2):
                        nc.tensor.matmul(
                            sc[:, h, qlo:qlo + 128],
                            idn_sb[:],
                            msk_sb[:],
                            start=False, stop=True,
                            skip_group_check=True,
                        )
                pr = prp.tile([128, 2, 512], f16, tag="pr", name="pr")
                if kb == 0 and cold:
                    # split per head so the first ctx matmul of the
                    # query tile is not gated on both heads' exp
                    for h in range(2):
                        nc.scalar.activation(
                            pr[:, h, qlo:512], sc[:, h, qlo:512],
                            AFT.Exp, scale=0.125
                        )
                else:
                    nc.scalar.activation(
                        pr[:, :, qlo:512], sc[:, :, qlo:512],
                        AFT.Exp, scale=0.125
                    )
                return (kb, nk, qlo, pr)

            def emit_cx(info, cx):
                kb, nk, qlo, pr = info
                st = kb == 0
                sp = kb == nk - 1
                nc.tensor.matmul(
                    cx[0:65, qlo:512], v0[:, kb, :], pr[:, 0, qlo:512],
                    start=st, stop=sp, skip_group_check=True,
                )
                nc.tensor.matmul(
                    cx[0:128, 512 + qlo:1024], v1e[:, kb, :], pr[:, 1, qlo:512],
                    start=st, stop=sp, skip_group_check=True,
                )

            def make_norm(tl, cx, q0):
                ctx = tl["ctx"]

                def norm(act_bc=False):
                    # denoms: h0 at cx[64, bankA], h1 at cx[32, bankB].
                    # reciprocal_approx_fast only works on full-width
                    # offset-0 tiles, so stage the two denom rows into
                    # an sbuf tile and reciprocate the whole tile
                    # (garbage rows are never read).
                    rci = nrmp.tile([128, 512], f32, tag="rci", name="rci")
                    nc.vector.tensor_copy(rci[64:65, :], cx[64:65, 0:512])
                    nc.vector.tensor_copy(rci[32:33, :], cx[32:33, 512:1024])
                    rc = nrmp.tile([128, 512], f32, tag="rc", name="rc")
                    nc.vector.reciprocal_approx_fast(rc[:], rci[:])
                    # bf16 view of rc's truncated high half-words:
                    # f32 bits[31:16] == bf16 round-toward-zero
                    rcb = rc.bitcast(bf16).rearrange(
                        "p (a two) -> p a two", two=2
                    )[:, :, 1]
                    # rank-1 broadcast into free psum rows of cx
                    nc.tensor.matmul(
                        cx[0:64, 512:1024], onr_sb[64:65, :], rcb[64:65, :],
                        start=True, stop=True, skip_group_check=True,
                    )
                    nc.tensor.matmul(
                        cx[64:128, 0:512], onr_sb[32:33, :], rcb[32:33, :],
                        start=True, stop=True, skip_group_check=True,
                    )
                    bc = nrmp.tile([128, 512], f32, tag="bc", name="bc")
                    if act_bc:
                        # flushes outside attention: ACT has no exp
                        # backlog and DVE is busy with outproj casts
                        nc.scalar.copy(bc[0:64, :], cx[0:64, 512:1024])
                        nc.scalar.copy(bc[64:128, :], cx[64:128, 0:512])
                    else:
                        nc.vector.tensor_copy(bc[0:64, :], cx[0:64, 512:1024])
                        nc.vector.tensor_copy(bc[64:128, :], cx[64:128, 0:512])
                    nc.vector.tensor_mul(
                        ctx[0:64, q0:q0 + 512], cx[0:64, 0:512],
                        bc[0:64, :]
                    )
                    nc.vector.tensor_mul(
                        ctx[64:128, q0:q0 + 512], cx[64:128, 512:1024],
                        bc[64:128, :]
                    )
                return norm

            def emit_attn_batch(b, tl, fillers, pend):
                """All 4 query tiles of batch b as one flat (qt, kb) stream.
                fillers: {qt: [(min_kb, closure), ...]} — one popped per step
                between sc(i) and cx(i-1); all remaining drained on the last
                step of the qt."""
                steps = [(qt, kb) for qt in range(_NQT)
                         for kb in range(4 * (qt + 1))]
                fillers = {qt: sorted(fq, key=lambda t: t[0])
                           for qt, fq in fillers.items()}
                cxs = {}
                prev = None
                for qt, kb in steps:
                    nk = 4 * (qt + 1)
                    q0 = qt * 512
                    if kb == 0:
                        cxs[qt] = cxp.tile([128, 1024], f32, tag="cx", name="cx")
                    info = emit_sc(tl, qt, kb, nk, q0, b == 0 and qt == 0)
                    fq = fillers.get(qt)
                    if fq:
                        if kb == nk - 1:
                            while fq:
                                fq.pop(0)[1]()
                        elif fq[0][0] <= kb:
                            fq.pop(0)[1]()
                    if prev is not None:
                        pinfo, pqt = prev
                        emit_cx(pinfo, cxs[pqt])
                        if pinfo[0] == pinfo[1] - 1:  # last kb: qt finished
                            pend[0] = make_norm(tl, cxs[pqt], pqt * 512)
                    prev = (info, qt)
                    if kb == 2 and pend[0] is not None:
                        pend[0]()
                        pend[0] = None
                emit_cx(prev[0], cxs[prev[1]])
                pend[0] = make_norm(tl, cxs[prev[1]], prev[1] * 512)

            # ---- schedule ----
            alloc_batch(0)
            emit_proj_unit(0, 0, wv_sb, bv_sb, "vT", is_v=True)
            emit_proj_unit(0, 0, wk_sb, bk_sb, "kT")
            emit_proj_unit(0, 0, wq_sb, bq_sb, "qT", is_q=True)

            pend = [None]
            for b in range(_B):
                F = {0: [], 1: [], 2: [], 3: []}
                # strip 1 of this batch feeds qt2/qt3; emit early in qt1
                F[1].append((0, lambda b=b: emit_proj_unit(
                    b, 1, wv_sb, bv_sb, "vT", is_v=True)))
                F[1].append((1, lambda b=b: emit_proj_unit(
                    b, 1, wk_sb, bk_sb, "kT")))
                F[1].append((2, lambda b=b: emit_proj_unit(
                    b, 1, wq_sb, bq_sb, "qT", is_q=True)))
                # own outproj, one query tile behind the norm
                for i, tkb in enumerate(range(0, 4)):
                    F[1].append((3 + i, lambda b=b, t=tkb: emit_outproj_unit(
                        b, t, "scalar" if t % 4 == 0 else "vector")))
                for i, tkb in enumerate(range(4, 8)):
                    F[2].append((4 + 2 * i, lambda b=b, t=tkb: emit_outproj_unit(
                        b, t, "scalar" if t % 4 == 0 else "vector")))
                for i, tkb in enumerate(range(8, 12)):
                    F[3].append((3 + 2 * i, lambda b=b, t=tkb: emit_outproj_unit(
                        b, t, "scalar" if t % 4 == 0 else "vector")))
                if b + 1 < _B:
                    F[2].append((1, lambda b=b: alloc_batch(b + 1)))
                    # next batch strip 0; min_kb=9 keeps the v0/v1e repack
                    # after this batch's qt3 ctx reads of key blocks 0-8
                    # (the filler at step kb runs before cx of step kb-1)
                    F[3].append((9, lambda b=b: emit_proj_unit(
                        b + 1, 0, wv_sb, bv_sb, "vT", is_v=True)))
                    F[3].append((11, lambda b=b: emit_proj_unit(
                        b + 1, 0, wk_sb, bk_sb, "kT")))
                    F[3].append((13, lambda b=b: emit_proj_unit(
                        b + 1, 0, wq_sb, bq_sb, "qT", is_q=True)))
                else:
                    # previous batch's last outproj group (needs the norm
                    # flushed at qt0 kb2)
                    for i, tkb in enumerate(range(12, 16)):
                        F[0].append((3, lambda b=b, t=tkb: emit_outproj_unit(
                            b - 1, t, "scalar" if t % 2 == 0 else "vector")))
                emit_attn_batch(b, tls[b], F, pend)

            # tail: last batch's final norm + last outproj group
            pend[0](True)
            pend[0] = None
            for tkb in range(12, 16):
                emit_outproj_unit(_B - 1, tkb,
                                  "scalar" if tkb % 2 == 0 else "vector")

    nc.compile()
    return nc


def _get_nc():
    if "nc" not in _cache:
        _cache["nc"] = _build_bass()
    return _cache["nc"]


def _host_inputs(x, Wq, bq, Wk, bk, Wv, bv, Wo, bo):
    x = np.asarray(x, np.float32)
    xT = np.ascontiguousarray(x.reshape(_T, _D).T.astype(np.float16))

    # additive causal mask for the diagonal 128x128 triangle: 0 where k<=c
    kk = np.arange(128)[:, None]
    cc = np.arange(128)[None, :]
    mskval = np.where(kk <= cc, 0.0, -1000.0).astype(np.float16)
    import ml_dtypes
    ident = np.eye(128, dtype=np.float16)
    ones = np.ones((128, _NKB), np.float16)
    onesr = np.ones((128, 64), ml_dtypes.bfloat16)
    zon = np.zeros((128, _NKB, 64), np.float16)
    zon[:, :, 32] = 1.0

    in_maps = []
    for c in range(_NC):
        s = slice(c * _DC, (c + 1) * _DC)
        in_maps.append({
            "xT": xT,
            "wq": np.ascontiguousarray(np.asarray(Wq, np.float32)[:, s].astype(np.float16)),
            "wk": np.ascontiguousarray(np.asarray(Wk, np.float32)[:, s].astype(np.float16)),
            "wv": np.ascontiguousarray(np.asarray(Wv, np.float32)[:, s].astype(np.float16)),
            "wo": np.ascontiguousarray(np.asarray(Wo, np.float32)[s, :].astype(np.float16)),
            "bq": np.ascontiguousarray(np.asarray(bq, np.float32)[s, None]),
            "bk": np.ascontiguousarray(np.asarray(bk, np.float32)[s, None]),
            "bv": np.ascontiguousarray(np.asarray(bv, np.float32)[s, None]),
            "idn": ident,
            "msk": mskval,
            "ons": ones,
            "onr": onesr,
            "zon": zon,
        })
    return in_maps


def kernel_run(x, Wq, bq, Wk, bk, Wv, bv, Wo, bo, trace=False):
    """Run the SPMD kernel; returns (full output, BassKernelResults)."""
    from concourse.bass_utils import run_bass_kernel_spmd

    nc = _get_nc()
    in_maps = _host_inputs(x, Wq, bq, Wk, bk, Wv, bv, Wo, bo)
    res = run_bass_kernel_spmd(nc, in_maps, list(range(_NC)), trace=trace)
    acc = np.zeros((_T, _D), np.float32)
    for c in range(_NC):
        acc += res.results[c]["out"]
    acc += np.asarray(bo, np.float32)[None, :]
    return acc.reshape(_B, _L, _D), res


def kernel(x, Wq, bq, Wk, bk, Wv, bv, Wo, bo):
    out, _ = kernel_run(x, Wq, bq, Wk, bk, Wv, bv, Wo, bo, trace=False)
    return out


# revision 11
# speedup vs baseline: 1.2065x; 1.2065x over previous
"""Causal multi-head attention on 8 trn2 NeuronCores (Megatron-style head parallelism).

Problem: B=2, L=2048, D=1024, H=16 heads (HD=64), fp32 in/out.

Sharding: each of the 8 cores owns 2 heads (a 128-wide slice of the QKV
projection output / Wo rows). Every core reads the full x; QKV projections are
column-sharded, attention runs per-head, the output projection is row-sharded
producing a partial sum per core which the host reduces (+ bo).

On-chip layout: activations are feature-major: x^T [D, B*L] (host
pre-transposes), Q^T/K^T/V^T [128(d), L] per batch. Scores are computed
transposed: S^T[k, q] = K_blk^T.T @ Q^T (contraction over head dim), exp on
the scalar engine, ctx^T[d, q] accumulates over key blocks with V-natural
(built via DMA-XBAR transpose into contiguous tiles) as the stationary
operand.

Perf structure (v2 — single interleaved PE stream):
  - everything is emitted as one continuous PE instruction stream per batch:
    attention steps carry "filler" work units (QKV projection / output
    projection matmul groups) injected between the score matmul of step i+1
    and the ctx matmul of step i.  The PE never waits on the scalar engine's
    exp (which covers its ~1us latency under the filler), stays p-state
    ramped, and the projection phases cost no extra wall-clock.
  - all PSUM work units allocate from the same 2-buf "sc" pool ring
    ([128,1024]f32 = 2 banks each; sc 4 banks + ctx 4 banks = 8 banks).
  - causal work trimmed at 128-col granularity on diagonal blocks; causal
    mask applied additively in PSUM via an identity-stationary matmul
    (value -1000 before the 1/8 softmax scale -> exp underflows to exact 0).
  - ctx packed [128d, L]: h0 ctx rows 0-64 of psum bank A (inline ones column
    gives the h0 softmax denominator in row 64), h1 ctx rows 64-127 of bank B
    with its denominator from a 1-col side-matmul into bank B row 32.
    Reciprocals via the fast DVE approx; the per-column broadcast is a rank-1
    f32r matmul into the ctx tile's free psum regions, emitted two score
    tiles into the next query tile so the PE never waits on the reciprocal.
  - output projection contracts all 128 dims in one matmul per (128-token
    block, half); interleaved into the SAME batch's attention one query tile
    behind the norm, so the tail is only the last query tile's outproj.
  - first x strip DMA'd in [64, 512] chunks ordered (half, ec, row) so the
    first projection matmul starts as soon as ~64KB has landed; weight DMAs
    split per 2 ec chunks.
"""

import numpy as np

_B, _L, _D, _H, _HD = 2, 2048, 1024, 16, 64
_NC = 8
_DC = _D // _NC          # 128 feature dims (2 heads) per core
_T = _B * _L             # 4096 tokens
_NKB = _L // 128         # 16 key blocks per batch
_NQT = _L // 512         # 4 query tiles per batch

_cache = {}


def _build_bass():
    from concourse import bacc
    import concourse.mybir as mybir
    import concourse.tile as tile

    f32 = mybir.dt.float32
    f16 = mybir.dt.float16
    bf16 = mybir.dt.bfloat16
    AFT = mybir.ActivationFunctionType

    nc = bacc.Bacc("TRN2", target_bir_lowering=False, debug=False, num_devices=_NC)

    xT = nc.dram_tensor("xT", [_D, _T], f16, kind="ExternalInput")
    wq = nc.dram_tensor("wq", [_D, _DC], f16, kind="ExternalInput")
    wk = nc.dram_tensor("wk", [_D, _DC], f16, kind="ExternalInput")
    wv = nc.dram_tensor("wv", [_D, _DC], f16, kind="ExternalInput")
    wo = nc.dram_tensor("wo", [_DC, _D], f16, kind="ExternalInput")
    bqd = nc.dram_tensor("bq", [_DC, 1], f32, kind="ExternalInput")
    bkd = nc.dram_tensor("bk", [_DC, 1], f32, kind="ExternalInput")
    bvd = nc.dram_tensor("bv", [_DC, 1], f32, kind="ExternalInput")
    idnd = nc.dram_tensor("idn", [128, 128], f16, kind="ExternalInput")
    mskd = nc.dram_tensor("msk", [128, 128], f16, kind="ExternalInput")
    onsd = nc.dram_tensor("ons", [128, _NKB], f16, kind="ExternalInput")
    zond = nc.dram_tensor("zon", [128, _NKB, 64], f16, kind="ExternalInput")
    onrd = nc.dram_tensor("onr", [128, 64], bf16, kind="ExternalInput")
    out = nc.dram_tensor("out", [_T, _D], f16, kind="ExternalOutput")

    with tile.TileContext(nc) as tc:
        with (
            tc.tile_pool(name="const", bufs=1) as constp,
            tc.tile_pool(name="xs", bufs=2) as xsp,
            tc.tile_pool(name="qkv", bufs=2) as qkvp,
            tc.tile_pool(name="pr", bufs=3) as prp,
            tc.tile_pool(name="nrm", bufs=2) as nrmp,
            tc.tile_pool(name="og", bufs=4) as ogp,
            tc.tile_pool(name="sc", bufs=2, space="PSUM") as scp,  # 2x[128,1024]f32 = 4 banks
            tc.tile_pool(name="cx", bufs=2, space="PSUM") as cxp,  # 2x[128,1024]f32 = 4 banks
        ):
            # ---- persistent constants ----
            # DMA emission order tracks first-use time: wv+bv, strip0 half 0
            # of x, wk, strip0 half 1, wq, then the rest.  The big strip-1
            # prefetch is emitted only after the strip-0 units so it queues
            # behind everything the head needs.
            wv_sb = constp.tile([128, 8, 128], f16, tag="wv")
            for i in range(4):
                nc.sync.dma_start(
                    wv_sb[:, 2 * i:2 * i + 2, :],
                    wv[2 * i * 128:(2 * i + 2) * 128, :].rearrange(
                        "(c p) d -> p c d", p=128
                    ),
                )
            bv_sb = constp.tile([128, 1], f32, tag="bv")
            nc.sync.dma_start(bv_sb[:], bvd[:])

            def prefetch_strip(xs, b, tb2, fine=False):
                c0 = b * _L + tb2 * 1024
                if fine:
                    # (half, ec) chunks so the V-proj matmuls start as soon
                    # as the first 128KB lands and pace with the DMA
                    for half in range(2):
                        for ec in range(8):
                            nc.sync.dma_start(
                                xs[:, ec, half * 512:(half + 1) * 512],
                                xT[ec * 128:(ec + 1) * 128,
                                   c0 + half * 512:c0 + (half + 1) * 512],
                            )
                else:
                    for i in range(2):
                        ec0 = i * 4
                        nc.sync.dma_start(
                            xs[:, ec0:ec0 + 4, :],
                            xT[ec0 * 128:(ec0 + 4) * 128,
                               c0:c0 + 1024].rearrange(
                                "(c p) t -> p c t", p=128
                            ),
                        )

            xs00 = xsp.tile([128, 8, 1024], f16, tag="xs", name="xs")
            wq_sb = constp.tile([128, 8, 128], f16, tag="wq")
            wk_sb = constp.tile([128, 8, 128], f16, tag="wk")
            bq_sb = constp.tile([128, 1], f32, tag="bq")
            bk_sb = constp.tile([128, 1], f32, tag="bk")

            prefetch_strip(xs00, 0, 0, fine=True)
            for i in range(4):
                nc.sync.dma_start(
                    wk_sb[:, 2 * i:2 * i + 2, :],
                    wk[2 * i * 128:(2 * i + 2) * 128, :].rearrange(
                        "(c p) d -> p c d", p=128
                    ),
                )
            nc.sync.dma_start(bk_sb[:], bkd[:])
            for i in range(4):
                nc.sync.dma_start(
                    wq_sb[:, 2 * i:2 * i + 2, :],
                    wq[2 * i * 128:(2 * i + 2) * 128, :].rearrange(
                        "(c p) d -> p c d", p=128
                    ),
                )
            nc.sync.dma_start(bq_sb[:], bqd[:])
            wo_sb = constp.tile([128, 1024], f16, tag="wo")
            nc.sync.dma_start(wo_sb[:], wo[:])
            idn_sb = constp.tile([128, 128], f16, tag="idn")
            nc.sync.dma_start(idn_sb[:], idnd[:])
            msk_sb = constp.tile([128, 128], f16, tag="msk")
            nc.sync.dma_start(msk_sb[:], mskd[:])
            ons_sb = constp.tile([128, _NKB], f16, tag="ons")
            nc.sync.dma_start(ons_sb[:], onsd[:])
            onr_sb = constp.tile([128, 64], bf16, tag="onr")
            nc.sync.dma_start(onr_sb[:], onrd[:])

            # persistent V stationaries. v0 = [V0 | ones]: ctx rows 0-63 +
            # h0 denom row 64. v1e = [0..0 | ones@32 | 0..0 | V1]: one fused
            # matmul yields h1 denom at row 32 and ctx at rows 64-127.
            # Constant columns are written once; V parts repacked per batch.
            v0 = qkvp.tile([128, _NKB, 65], f16, tag="v0", name="v0", bufs=1)
            v1e = qkvp.tile([128, _NKB, 128], f16, tag="v1e", name="v1e", bufs=1)
            nc.vector.tensor_copy(v0[:, :, 64], ons_sb[:])
            nc.sync.dma_start(v1e[:, :, 0:64], zond[:])

            tls = {}

            def alloc_batch(b):
                qT = qkvp.tile([128, _L], f16, tag="qT", name="qT")
                kT = qkvp.tile([128, _L], f16, tag="kT", name="kT")
                vT = qkvp.tile([128, _L], f16, tag="vT", name="vT", bufs=1)
                ctx = qkvp.tile([128, _L], f16, tag="ctx", name="ctx")
                # V natural via DMA XBAR transpose (contiguous dests only),
                # then DVE re-pack into the strided stationary tiles.
                v0t = qkvp.tile([128, _NKB, 64], f16, tag="v0t", name="v0t", bufs=1)
                v1t = qkvp.tile([128, _NKB, 64], f16, tag="v1t", name="v1t", bufs=1)
                if b == 0:
                    # strip 1's prefetch is emitted after the strip-0 units
                    xss = [xs00, xsp.tile([128, 8, 1024], f16, tag="xs", name="xs")]
                else:
                    xss = []
                    for tb2 in range(2):
                        xs = xsp.tile([128, 8, 1024], f16, tag="xs", name="xs")
                        prefetch_strip(xs, b, tb2)
                        xss.append(xs)
                tls[b] = dict(qT=qT, kT=kT, vT=vT, ctx=ctx, v0t=v0t, v1t=v1t,
                              xss=xss)
                return tls[b]

            def emit_proj_unit(b, tb2, w_sb, b_sb, dkey, is_v=False, is_q=False):
                """One strip x one projection: 16 matmuls + bias drain
                (+ V transpose/repack chain). ~3.4us of PE work."""
                tl = tls[b]
                xs = tl["xss"][tb2]
                dst = tl[dkey]
                ps = scp.tile([128, 1024], f32, tag="sc", name="pps")
                for half in range(2):
                    col = half * 512
                    for ec in range(8):
                        nc.tensor.matmul(
                            ps[:, col:col + 512],
                            w_sb[:, ec, :],
                            xs[:, ec, col:col + 512],
                            start=(ec == 0),
                            stop=(ec == 7),
                        )
                if is_q:
                    for bh in range(2):
                        bcol = tb2 * 1024 + bh * 512
                        nc.vector.tensor_scalar_add(
                            dst[:, bcol:bcol + 512],
                            ps[:, bh * 512:(bh + 1) * 512], b_sb[:]
                        )
                else:
                    nc.vector.tensor_scalar_add(
                        dst[:, tb2 * 1024:(tb2 + 1) * 1024], ps[:], b_sb[:]
                    )
                if is_v:
                    v0t, v1t = tl["v0t"], tl["v1t"]
                    cs = slice(tb2 * 1024, (tb2 + 1) * 1024)
                    kbs = slice(tb2 * 8, (tb2 + 1) * 8)
                    nc.sync.dma_start(
                        v0t[:, kbs, :], dst[0:64, cs], transpose=True
                    )
                    nc.sync.dma_start(
                        v1t[:, kbs, :], dst[64:128, cs], transpose=True
                    )
                    nc.vector.tensor_copy(v0[:, kbs, 0:64], v0t[:, kbs, :])
                    nc.vector.tensor_copy(v1e[:, kbs, 64:128], v1t[:, kbs, :])

            def emit_outproj_unit(b, tkb, eng):
                """Output projection for one 128-token block: 2 matmuls +
                staging copy + DMA. ~0.45us of PE work."""
                ctx = tls[b]["ctx"]
                op = scp.tile([128, 1024], f32, tag="sc", name="op")
                for half in range(2):
                    col = half * 512
                    nc.tensor.matmul(
                        op[:, col:col + 512],
                        ctx[:, tkb * 128:(tkb + 1) * 128],
                        wo_sb[:, col:col + 512],
                        start=True, stop=True,
                    )
                stg = ogp.tile([128, 1024], f16, tag="og", name="stg")
                if eng == "scalar":
                    nc.scalar.copy(stg[:], op[:])
                else:
                    nc.vector.tensor_copy(stg[:], op[:])
                r0 = b * _L + tkb * 128
                nc.sync.dma_start(out[r0:r0 + 128, :], stg[:])

            # ---- attention machinery ----
            def emit_sc(tl, qt, kb, nk, q0, cold):
                qT, kT = tl["qT"], tl["kT"]
                j = kb - (nk - 4)
                qlo = 128 * j if j > 0 else 0
                sc = scp.tile([128, 2, 512], f32, tag="sc", name="sct")
                for h in range(2):
                    hp = h * 64
                    nc.tensor.matmul(
                        sc[:, h, qlo:512],
                        kT[hp:hp + 64, kb * 128:(kb + 1) * 128],
                        qT[hp:hp + 64, q0 + qlo:q0 + 512],
                        start=True, stop=(j < 0),
                    )
                if j >= 0:
                    for h in range(2):
                        nc.tensor.matmul(
                            sc[:, h, qlo:qlo + 128],
                            idn_sb[:],
                            msk_sb[:],
                            start=False, stop=True,
                            skip_group_check=True,
                        )
                pr = prp.tile([128, 2, 512], f16, tag="pr", name="pr")
                if kb == 0 and cold:
                    # split per head so the first ctx matmul of the
                    # query tile is not gated on both heads' exp
                    for h in range(2):
                        nc.scalar.activation(
                            pr[:, h, qlo:512], sc[:, h, qlo:512],
                            AFT.Exp, scale=0.125
                        )
                else:
                    nc.scalar.activation(
                        pr[:, :, qlo:512], sc[:, :, qlo:512],
                        AFT.Exp, scale=0.125
                    )
                return (kb, nk, qlo, pr)

            def emit_cx(info, cx):
                kb, nk, qlo, pr = info
                st = kb == 0
                sp = kb == nk - 1
                nc.tensor.matmul(
                    cx[0:65, qlo:512], v0[:, kb, :], pr[:, 0, qlo:512],
                    start=st, stop=sp, skip_group_check=True,
                )
                nc.tensor.matmul(
                    cx[0:128, 512 + qlo:1024], v1e[:, kb, :], pr[:, 1, qlo:512],
                    start=st, stop=sp, skip_group_check=True,
                )

            def make_norm(tl, cx, q0):
                ctx = tl["ctx"]

                def norm(act_bc=False):
                    # denoms: h0 at cx[64, bankA], h1 at cx[32, bankB].
                    # reciprocal_approx_fast only works on full-width
                    # offset-0 tiles, so stage the two denom rows into
                    # an sbuf tile and reciprocate the whole tile
                    # (garbage rows are never read).
                    rci = nrmp.tile([128, 512], f32, tag="rci", name="rci")
                    nc.vector.tensor_copy(rci[64:65, :], cx[64:65, 0:512])
                    nc.vector.tensor_copy(rci[32:33, :], cx[32:33, 512:1024])
                    rc = nrmp.tile([128, 512], f32, tag="rc", name="rc")
                    nc.vector.reciprocal_approx_fast(rc[:], rci[:])
                    # bf16 view of rc's truncated high half-words:
                    # f32 bits[31:16] == bf16 round-toward-zero
                    rcb = rc.bitcast(bf16).rearrange(
                        "p (a two) -> p a two", two=2
                    )[:, :, 1]
                    # rank-1 broadcast into free psum rows of cx
                    nc.tensor.matmul(
                        cx[0:64, 512:1024], onr_sb[64:65, :], rcb[64:65, :],
                        start=True, stop=True, skip_group_check=True,
                    )
                    nc.tensor.matmul(
                        cx[64:128, 0:512], onr_sb[32:33, :], rcb[32:33, :],
                        start=True, stop=True, skip_group_check=True,
                    )
                    bc = nrmp.tile([128, 512], f32, tag="bc", name="bc")
                    if act_bc:
                        # flushes outside attention: ACT has no exp
                        # backlog and DVE is busy with outproj casts
                        nc.scalar.copy(bc[0:64, :], cx[0:64, 512:1024])
                        nc.scalar.copy(bc[64:128, :], cx[64:128, 0:512])
                    else:
                        nc.vector.tensor_copy(bc[0:64, :], cx[0:64, 512:1024])
                        nc.vector.tensor_copy(bc[64:128, :], cx[64:128, 0:512])
                    nc.vector.tensor_mul(
                        ctx[0:64, q0:q0 + 512], cx[0:64, 0:512],
                        bc[0:64, :]
                    )
                    nc.vector.tensor_mul(
                        ctx[64:128, q0:q0 + 512], cx[64:128, 512:1024],
                        bc[64:128, :]
                    )
                return norm

            def emit_attn_batch(b, tl, fillers, pend):
                """All 4 query tiles of batch b as one flat (qt, kb) stream.
                fillers: {qt: [(min_kb, closure), ...]} — one popped per step
                between sc(i) and cx(i-1); all remaining drained on the last
                step of the qt."""
                steps = [(qt, kb) for qt in range(_NQT)
                         for kb in range(4 * (qt + 1))]
                fillers = {qt: sorted(fq, key=lambda t: t[0])
                           for qt, fq in fillers.items()}
                cxs = {}
                prev = None
                for qt, kb in steps:
                    nk = 4 * (qt + 1)
                    q0 = qt * 512
                    if kb == 0:
                        cxs[qt] = cxp.tile([128, 1024], f32, tag="cx", name="cx")
                    info = emit_sc(tl, qt, kb, nk, q0, b == 0 and qt == 0)
                    fq = fillers.get(qt)
                    if fq:
                        if kb == nk - 1:
                            while fq:
                                fq.pop(0)[1]()
                        elif fq[0][0] <= kb:
                            fq.pop(0)[1]()
                    if prev is not None:
                        pinfo, pqt = prev
                        emit_cx(pinfo, cxs[pqt])
                        if pinfo[0] == pinfo[1] - 1:  # last kb: qt finished
                            pend[0] = make_norm(tl, cxs[pqt], pqt * 512)
                    prev = (info, qt)
                    if kb == 2 and pend[0] is not None:
                        pend[0]()
                        pend[0] = None
                emit_cx(prev[0], cxs[prev[1]])
                pend[0] = make_norm(tl, cxs[prev[1]], prev[1] * 512)

            # ---- schedule ----
            alloc_batch(0)
            emit_proj_unit(0, 0, wv_sb, bv_sb, "vT", is_v=True)
            emit_proj_unit(0, 0, wk_sb, bk_sb, "kT")
            emit_proj_unit(0, 0, wq_sb, bq_sb, "qT", is_q=True)
            prefetch_strip(tls[0]["xss"][1], 0, 1)

            # Outproj placement rule: tg_i's norm flushes at (qt_{i+1}, kb2)
            # and its DVE chain settles ~3.5us later, so tg_i units go one
            # full query tile later still (or late in qt_{i+1}).
            def op_unit(b, tkb):
                return lambda: emit_outproj_unit(
                    b, tkb, "scalar" if tkb % 4 == 0 else "vector")

            pend = [None]
            for b in range(_B):
                last = b + 1 == _B
                F = {0: [], 1: [], 2: [], 3: []}
                # strip 1 of this batch feeds qt2/qt3; emit early in qt1
                F[1].append((0, lambda b=b: emit_proj_unit(
                    b, 1, wv_sb, bv_sb, "vT", is_v=True)))
                F[1].append((1, lambda b=b: emit_proj_unit(
                    b, 1, wk_sb, bk_sb, "kT")))
                F[1].append((2, lambda b=b: emit_proj_unit(
                    b, 1, wq_sb, bq_sb, "qT", is_q=True)))
                if b > 0:
                    # previous batch's tg2 (norms long settled) and tg3
                    # (norm flushed at qt0 kb2, settled by qt1)
                    F[0].append((2, op_unit(b - 1, 8)))
                    F[0].append((3, op_unit(b - 1, 9)))
                    F[2].append((2, op_unit(b - 1, 10)))
                    F[2].append((3, op_unit(b - 1, 11)))
                    for i, tkb in enumerate(range(12, 16)):
                        F[1].append((3 + i, op_unit(b - 1, tkb)))
                for i, tkb in enumerate(range(0, 4)):
                    F[2].append((6 + i, op_unit(b, tkb)))
                for i, tkb in enumerate(range(4, 8)):
                    F[3].append((3 + 2 * i, op_unit(b, tkb)))
                if b + 1 < _B:
                    F[2].append((1, lambda b=b: alloc_batch(b + 1)))
                    # next batch strip 0; min_kb=10 keeps the v0/v1e repack
                    # after this batch's qt3 ctx reads of key blocks 0-8
                    # (the filler at step kb runs before cx of step kb-1)
                    F[3].append((10, lambda b=b: emit_proj_unit(
                        b + 1, 0, wv_sb, bv_sb, "vT", is_v=True)))
                    F[3].append((12, lambda b=b: emit_proj_unit(
                        b + 1, 0, wk_sb, bk_sb, "kT")))
                    F[3].append((14, lambda b=b: emit_proj_unit(
                        b + 1, 0, wq_sb, bq_sb, "qT", is_q=True)))
                else:
                    for i, tkb in enumerate(range(8, 12)):
                        F[3].append((9 + 2 * i, op_unit(b, tkb)))
                emit_attn_batch(b, tls[b], F, pend)

            # tail: last batch's final norm + last outproj group
            pend[0](True)
            pend[0] = None
            for tkb in range(12, 16):
                emit_outproj_unit(_B - 1, tkb,
                                  "scalar" if tkb % 2 == 0 else "vector")

    nc.compile()
    return nc


def _get_nc():
    if "nc" not in _cache:
        _cache["nc"] = _build_bass()
    return _cache["nc"]


def _host_inputs(x, Wq, bq, Wk, bk, Wv, bv, Wo, bo):
    x = np.asarray(x, np.float32)
    xT = np.ascontiguousarray(x.reshape(_T, _D).T.astype(np.float16))

    # additive causal mask for the diagonal 128x128 triangle: 0 where k<=c
    kk = np.arange(128)[:, None]
    cc = np.arange(128)[None, :]
    mskval = np.where(kk <= cc, 0.0, -1000.0).astype(np.float16)
    import ml_dtypes
    ident = np.eye(128, dtype=np.float16)
    ones = np.ones((128, _NKB), np.float16)
    onesr = np.ones((128, 64), ml_dtypes.bfloat16)
    zon = np.zeros((128, _NKB, 64), np.float16)
    zon[:, :, 32] = 1.0

    in_maps = []
    for c in range(_NC):
        s = slice(c * _DC, (c + 1) * _DC)
        in_maps.append({
            "xT": xT,
            "wq": np.ascontiguousarray(np.asarray(Wq, np.float32)[:, s].astype(np.float16)),
            "wk": np.ascontiguousarray(np.asarray(Wk, np.float32)[:, s].astype(np.float16)),
            "wv": np.ascontiguousarray(np.asarray(Wv, np.float32)[:, s].astype(np.float16)),
            "wo": np.ascontiguousarray(np.asarray(Wo, np.float32)[s, :].astype(np.float16)),
            "bq": np.ascontiguousarray(np.asarray(bq, np.float32)[s, None]),
            "bk": np.ascontiguousarray(np.asarray(bk, np.float32)[s, None]),
            "bv": np.ascontiguousarray(np.asarray(bv, np.float32)[s, None]),
            "idn": ident,
            "msk": mskval,
            "ons": ones,
            "onr": onesr,
            "zon": zon,
        })
    return in_maps


def kernel_run(x, Wq, bq, Wk, bk, Wv, bv, Wo, bo, trace=False):
    """Run the SPMD kernel; returns (full output, BassKernelResults)."""
    from concourse.bass_utils import run_bass_kernel_spmd

    nc = _get_nc()
    in_maps = _host_inputs(x, Wq, bq, Wk, bk, Wv, bv, Wo, bo)
    res = run_bass_kernel_spmd(nc, in_maps, list(range(_NC)), trace=trace)
    acc = np.zeros((_T, _D), np.float32)
    for c in range(_NC):
        acc += res.results[c]["out"]
    acc += np.asarray(bo, np.float32)[None, :]
    return acc.reshape(_B, _L, _D), res


def kernel(x, Wq, bq, Wk, bk, Wv, bv, Wo, bo):
    out, _ = kernel_run(x, Wq, bq, Wk, bk, Wv, bv, Wo, bo, trace=False)
    return out
